# revision 3
# baseline (speedup 1.0000x reference)
"""ECPGLinear (ternary-quantized linear) Bass kernel for 8 TRN2 NeuronCores.

Computes out = x @ W.T where W = dequant(ternary, per-group scales),
group_size=128 along in_features.

Sharding: data-parallel over the 8192 (batch*seq) tokens — each core gets
1024 rows of x, full dequantized weights are (re)built on-device per core.
No collectives; host concatenates the 8 output shards.

Device math per core (all on-device):
  - X^T shard resident in SBUF (fp32, used as float32r by the PE).
  - ternary^T streamed as fp32 tiles; scales broadcast across partitions
    via GPSIMD partition_broadcast; DVE multiplies -> dequantized W^T tile.
  - PE accumulates out[m,o] = sum_k XT[k,m]^T WT[k,o] over 32 k-tiles in
    PSUM (float32r matmuls: bf16-rate streaming with ~1.5e-4 rel err).
"""
import functools
import numpy as np

OUT_F = 4096
IN_F = 4096
GS = 128
B, S = 4, 2048
M_TOT = B * S            # 8192 tokens
NCORES = 8
M_CORE = M_TOT // NCORES  # 1024 tokens per core
KT = IN_F // 128          # 32 contraction tiles
NCH = OUT_F // 512        # 8 output chunks of 512
MT = M_CORE // 128        # 8 m-tiles per core


@functools.lru_cache(maxsize=1)
def _build():
    from concourse import bacc
    import concourse.mybir as mybir
    import concourse.tile as tile

    f32 = mybir.dt.float32
    f32r = mybir.dt.float32r

    nc = bacc.Bacc("TRN2", target_bir_lowering=False, debug=False,
                   num_devices=NCORES)
    xt = nc.dram_tensor("xt", [IN_F, M_CORE], f32r, kind="ExternalInput")
    tt = nc.dram_tensor("tt", [IN_F, OUT_F], f32, kind="ExternalInput")
    sc = nc.dram_tensor("sc", [KT, OUT_F], f32, kind="ExternalInput")
    out = nc.dram_tensor("out", [M_CORE, OUT_F], f32, kind="ExternalOutput")

    with tile.TileContext(nc) as tc:
        with (
            tc.tile_pool(name="xres", bufs=1) as xres_pool,
            tc.tile_pool(name="scst", bufs=4) as scst_pool,
            tc.tile_pool(name="scb", bufs=4) as scb_pool,
            tc.tile_pool(name="tern", bufs=4) as tern_pool,
            tc.tile_pool(name="wdeq", bufs=4) as wdeq_pool,
            tc.tile_pool(name="ost", bufs=8) as ost_pool,
            tc.tile_pool(name="psum", bufs=8, space="PSUM") as psum_pool,
        ):
            # Resident X^T: [128 part, KT, M_CORE]
            xres = xres_pool.tile([128, KT, M_CORE], f32r)
            for kt in range(KT):
                nc.sync.dma_start(xres[:, kt, :], xt[kt * 128:(kt + 1) * 128, :])

            for n in range(NCH):
                psums = [psum_pool.tile([128, 512], f32, name=f"ps{n}_{m}",
                                        tag="ps")
                         for m in range(MT)]
                for kt in range(KT):
                    scst = scst_pool.tile([1, 512], f32)
                    nc.sync.dma_start(
                        scst[:], sc[kt:kt + 1, n * 512:(n + 1) * 512])
                    scb = scb_pool.tile([128, 512], f32)
                    nc.gpsimd.partition_broadcast(scb[:], scst[:])
                    tern = tern_pool.tile([128, 512], f32)
                    nc.sync.dma_start(
                        tern[:],
                        tt[kt * 128:(kt + 1) * 128, n * 512:(n + 1) * 512])
                    wdeq = wdeq_pool.tile([128, 512], f32r)
                    nc.vector.tensor_mul(wdeq[:], tern[:], scb[:])
                    wr = wdeq[:]
                    for m in range(MT):
                        nc.tensor.matmul(
                            psums[m][:],
                            xres[:, kt, m * 128:(m + 1) * 128],
                            wr,
                            start=(kt == 0),
                            stop=(kt == KT - 1),
                        )
                for m in range(MT):
                    ost = ost_pool.tile([128, 512], f32)
                    nc.scalar.copy(ost[:], psums[m][:])
                    nc.gpsimd.dma_start(
                        out[m * 128:(m + 1) * 128, n * 512:(n + 1) * 512],
                        ost[:])

    nc.compile()
    return nc


def kernel(x: np.ndarray, ternary: np.ndarray, scales: np.ndarray,
           _trace: bool = False):
    from concourse.bass_utils import run_bass_kernel_spmd

    nc = _build()

    xf = np.ascontiguousarray(x.reshape(M_TOT, IN_F))
    # ternary^T in fp32: [IN_F, OUT_F]; values in {-1, 0, 1} (exact)
    ttm = np.ascontiguousarray(ternary.T.astype(np.float32))
    # scales rearranged to [KT, OUT_F]: sc[kt, o] = scales[o*KT + kt]
    scm = np.ascontiguousarray(scales.reshape(OUT_F, KT).T.astype(np.float32))

    in_maps = []
    for c in range(NCORES):
        xc = np.ascontiguousarray(
            xf[c * M_CORE:(c + 1) * M_CORE, :].T)  # [IN_F, M_CORE]
        in_maps.append({"xt": xc, "tt": ttm, "sc": scm})

    res = run_bass_kernel_spmd(nc, in_maps, list(range(NCORES)),
                               trace=_trace)
    outs = [res.results[c]["out"] for c in range(NCORES)]
    full = np.concatenate(outs, axis=0).reshape(B, S, OUT_F)
    if _trace:
        kernel.last_results = res
    return full


kernel.last_results = None
